# revision 21
# baseline (speedup 1.0000x reference)
"""Trainium2 kernel for per-class conditional dense (MoE-style routing).

    out[b] = x[b] @ W[classes[b]] + bias[classes[b]]
    x: [2048, 512] f32, classes: [2048, 1] int, W: [100, 512, 512] f32,
    bias: [100, 512] f32 -> out: [2048, 512] f32

Sharding: expert-parallel across 8 NeuronCores (grouped-GEMM style).
Class c is owned by core c // 13 (13 class slots per core; 8*13 = 104
slots cover the 100 classes, the last 4 slots are dummies). The host
routes each sample to the core owning its class, packing the samples of
each class into a fixed-width (S columns, zero-padded) block of a
transposed activation panel xt = [D, 13*S]. Each core then runs a fully
static grouped GEMM over its 13 slots:

    outT[u, col] = sum_d W_slot[d, u] * xt[d, col]   (+ bias, fused)

so each core reads only its own ~13 MB slice of the 100 MB weight table
exactly once (the memory roofline for this op), and the host scatters
the transposed result panels back to the original row order.
"""

import numpy as np

try:
    import concourse.bass as bass
except ImportError:  # pragma: no cover - fallback for bare environments
    import sys

    for _p in ("/opt/trn_rl_repo", "/root/.axon_site/_ro/trn_rl_repo"):
        if _p not in sys.path:
            sys.path.insert(0, _p)
    import concourse.bass as bass

import bass_rust
import concourse.tile as tile
from concourse import mybir
from concourse.bass_utils import run_bass_kernel_spmd

B, D, U, C = 2048, 512, 512, 100
NCORES = 8
CPC = 13  # class slots per core (8 * 13 = 104 >= C)
PT = 128  # partition tile
KT = D // PT  # contraction-dim tiles
UT = U // PT  # output-partition tiles

_PROG_CACHE = {}
LAST_RESULTS = None  # BassKernelResults of the most recent device run


def _split_multi_waits(nc):
    """Walrus on this image only accepts one sync wait per instruction.

    Tile emits multi-wait instructions (notably the kernel-tail Drain,
    which waits on every live semaphore). Split each extra wait onto a
    same-engine NoOp inserted immediately before the instruction.
    """
    for fn in nc.m.functions:
        for bb in fn.blocks:
            new = []
            changed = False
            for inst in bb.instructions:
                si = inst.sync_info
                waits = list(si.on_wait) if si else []
                if len(waits) > 1:
                    for idx, w in enumerate(waits[:-1]):
                        nop = mybir.InstNoOp(
                            name=f"{inst.name}-waitsplit{idx}", ins=[], outs=[]
                        )
                        nop.engine = inst.engine
                        nop.sync_info = bass_rust.SyncInfo(
                            on_wait=[w], on_update=[]
                        )
                        new.append(nop)
                    inst.sync_info = bass_rust.SyncInfo(
                        on_wait=[waits[-1]], on_update=list(si.on_update)
                    )
                    changed = True
                new.append(inst)
            if changed:
                bb.instructions = new


def _build_program(S):
    """One SPMD program, shared by all 8 cores; per-core data differs.

    Per core: xt [D, 13*S] (transposed, class-blocked activations),
    wt [13, D, U] (this core's weight slots) -> out [13*S, U].

    The x panel is the PE-stationary operand (it is tiny) and the weight
    rows stream through as the moving operand in float32r, which runs at
    full rate for moving dims >= 256 (plain fp32 matmul is 4 cycles/row
    and lowers to two LDWEIGHTS+MATMUL pairs).
    """
    f32 = mybir.dt.float32
    f32r = mybir.dt.float32r
    NCOL = CPC * S

    nc = bass.Bass()
    xt = nc.dram_tensor("xt", [D, NCOL], f32, kind="ExternalInput")
    wt = nc.dram_tensor("wt", [CPC, D, U], f32, kind="ExternalInput")
    out = nc.dram_tensor("out", [NCOL, U], f32, kind="ExternalOutput")

    # Output rows are packed 4 class slots (4*S=128 rows) per SBUF tile.
    OG = -(-CPC // 4)  # output groups

    with tile.TileContext(nc) as tc:
        with (
            tc.tile_pool(name="xp", bufs=1) as xp,
            tc.tile_pool(name="wp", bufs=CPC) as wp,
            tc.tile_pool(name="op", bufs=1) as op,
            tc.tile_pool(name="pp", bufs=4, space="PSUM") as pp,
            tc.tile_pool(name="ap", bufs=1, space="PSUM") as apool,
        ):
            # Single DMA for all of xT: [D, NCOL] -> [128, KT*NCOL].
            xt_t = xp.tile([PT, KT * NCOL], f32, name="x")
            nc.sync.dma_start(
                xt_t[:].rearrange("p (i c) -> p i c", i=KT),
                xt.rearrange("(i p) c -> p i c", p=PT),
            )

            # Per-class weight DMA: [D, U] -> [128, KT*U] (column block i
            # holds rows 128i..128(i+1) of W), double-buffered CPC deep.
            w_ts = []
            for j in range(CPC):
                w_t = wp.tile([PT, KT * U], f32, tag="w", name=f"w{j}")
                nc.sync.dma_start(
                    w_t[:].rearrange("p (i u) -> p i u", i=KT),
                    wt[j].rearrange("(i p) u -> p i u", p=PT),
                )
                w_ts.append(w_t)

            ots = [op.tile([PT, U], f32, name=f"o{g}") for g in range(OG)]

            # The LDWEIGHTS half of a matmul only supports one sync wait.
            # Per class, a 1x1 dummy matmul reading only w_t absorbs the
            # weight-DMA wait on the PE engine, so the real matmuls below
            # carry at most the PSUM-reuse wait. The dummies form one
            # accumulation group into the same scratch element so they do
            # not create PSUM WAW waits among themselves.
            scrps = apool.tile([1, 1], f32, name="abs")
            for j in range(CPC):
                nc.tensor.matmul(
                    scrps[:],
                    w_ts[j][:, 0:1],
                    w_ts[j][:, 1:2],
                    start=(j == 0),
                    stop=(j == CPC - 1),
                    skip_group_check=True,
                )
                ps = pp.tile([S, U], f32, tag="ps", name=f"ps{j}")
                for i in range(KT):
                    nc.tensor.matmul(
                        ps[:],
                        xt_t[:, i * NCOL + S * j : i * NCOL + S * (j + 1)],
                        w_ts[j][:, i * U : (i + 1) * U],
                        start=(i == 0),
                        stop=(i == KT - 1),
                    )
                # PSUM -> SBUF drain into the packed output tile.
                nc.scalar.copy(ots[j // 4][S * (j % 4) : S * (j % 4) + S, :], ps[:])

            for g in range(OG):
                rows = min(PT, NCOL - PT * g)
                nc.sync.dma_start(out[PT * g : PT * g + rows, :], ots[g][:rows, :])
    _split_multi_waits(nc)
    return nc


def kernel(x, classes, kernel, bias):
    global LAST_RESULTS
    x = np.ascontiguousarray(np.asarray(x), dtype=np.float32)
    W = np.ascontiguousarray(np.asarray(kernel), dtype=np.float32)
    bias_np = np.ascontiguousarray(np.asarray(bias), dtype=np.float32)
    cls = np.asarray(classes).reshape(-1).astype(np.int64)

    counts = np.bincount(cls, minlength=C)
    # Fixed column width per class slot; multiple of 8 for DMA alignment.
    S = int(max(32, -(-counts.max() // 8) * 8))
    if S not in _PROG_CACHE:
        _PROG_CACHE[S] = _build_program(S)
    nc = _PROG_CACHE[S]
    NCOL = CPC * S

    order = np.argsort(cls, kind="stable")
    starts = np.zeros(C + 1, np.int64)
    np.cumsum(counts[:C], out=starts[1:])
    rows_by_class = [order[starts[c] : starts[c + 1]] for c in range(C)]

    in_maps = []
    for m in range(NCORES):
        xt_m = np.zeros((D, NCOL), np.float32)
        for j in range(CPC):
            c = m * CPC + j
            if c >= C:
                continue
            r = rows_by_class[c]
            if r.size:
                xt_m[:, S * j : S * j + r.size] = x[r].T
        if (m + 1) * CPC <= C:
            wt_m = W[m * CPC : (m + 1) * CPC]
        else:
            n_real = C - m * CPC
            wt_m = np.concatenate([W[m * CPC : C], W[: CPC - n_real]], axis=0)
        in_maps.append({"xt": xt_m, "wt": wt_m})

    res = run_bass_kernel_spmd(nc, in_maps, list(range(NCORES)))
    LAST_RESULTS = res

    out = np.empty((B, U), np.float32)
    for m in range(NCORES):
        panel = np.asarray(res.results[m]["out"])
        for j in range(CPC):
            c = m * CPC + j
            if c >= C:
                continue
            r = rows_by_class[c]
            if r.size:
                out[r] = panel[S * j : S * j + r.size] + bias_np[c]
    return out


# revision 24
# speedup vs baseline: 1.2042x; 1.2042x over previous
"""Trainium2 kernel for per-class conditional dense (MoE-style routing).

    out[b] = x[b] @ W[classes[b]] + bias[classes[b]]
    x: [2048, 512] f32, classes: [2048, 1] int, W: [100, 512, 512] f32,
    bias: [100, 512] f32 -> out: [2048, 512] f32

Sharding: expert-parallel across 8 NeuronCores (grouped-GEMM style).
Class c is owned by core c // 13 (13 class slots per core; 8*13 = 104
slots cover the 100 classes, the last 4 slots are dummies). The host
routes each sample to the core owning its class, packing the samples of
each class into a fixed-width (S columns, zero-padded) block of a
transposed activation panel xt = [D, 13*S]. Each core then runs a fully
static grouped GEMM over its 13 slots:

    outT[u, col] = sum_d W_slot[d, u] * xt[d, col]   (+ bias, fused)

so each core reads only its own ~13 MB slice of the 100 MB weight table
exactly once (the memory roofline for this op), and the host scatters
the transposed result panels back to the original row order.
"""

import numpy as np

try:
    import concourse.bass as bass
except ImportError:  # pragma: no cover - fallback for bare environments
    import sys

    for _p in ("/opt/trn_rl_repo", "/root/.axon_site/_ro/trn_rl_repo"):
        if _p not in sys.path:
            sys.path.insert(0, _p)
    import concourse.bass as bass

import bass_rust
import concourse.tile as tile
from concourse import mybir
from concourse.bass_utils import run_bass_kernel_spmd

B, D, U, C = 2048, 512, 512, 100
NCORES = 8
CPC = 13  # class slots per core (8 * 13 = 104 >= C)
PT = 128  # partition tile
KT = D // PT  # contraction-dim tiles
UT = U // PT  # output-partition tiles

_PROG_CACHE = {}
LAST_RESULTS = None  # BassKernelResults of the most recent device run


def _split_multi_waits(nc):
    """Walrus on this image only accepts one sync wait per instruction.

    Tile emits multi-wait instructions (notably the kernel-tail Drain,
    which waits on every live semaphore). Split each extra wait onto a
    same-engine NoOp inserted immediately before the instruction.
    """
    for fn in nc.m.functions:
        for bb in fn.blocks:
            new = []
            changed = False
            for inst in bb.instructions:
                si = inst.sync_info
                waits = list(si.on_wait) if si else []
                if len(waits) > 1:
                    for idx, w in enumerate(waits[:-1]):
                        nop = mybir.InstNoOp(
                            name=f"{inst.name}-waitsplit{idx}", ins=[], outs=[]
                        )
                        nop.engine = inst.engine
                        nop.sync_info = bass_rust.SyncInfo(
                            on_wait=[w], on_update=[]
                        )
                        new.append(nop)
                    inst.sync_info = bass_rust.SyncInfo(
                        on_wait=[waits[-1]], on_update=list(si.on_update)
                    )
                    changed = True
                new.append(inst)
            if changed:
                bb.instructions = new


def _build_program(S):
    """One SPMD program, shared by all 8 cores; per-core data differs.

    Per core: xt [D, 13*S] (transposed, class-blocked activations),
    wt [13, D, U] (this core's weight slots) -> out [13*S, U].

    The x panel is the PE-stationary operand (it is tiny) and the weight
    rows stream through as the moving operand in float32r, which runs at
    full rate for moving dims >= 256 (plain fp32 matmul is 4 cycles/row
    and lowers to two LDWEIGHTS+MATMUL pairs).
    """
    f32 = mybir.dt.float32
    bf16 = mybir.dt.bfloat16
    NCOL = CPC * S

    nc = bass.Bass()
    xt = nc.dram_tensor("xt", [D, NCOL], f32, kind="ExternalInput")
    wt = nc.dram_tensor("wt", [CPC, D, U], f32, kind="ExternalInput")
    out = nc.dram_tensor("out", [NCOL, U], f32, kind="ExternalOutput")

    # Output rows are packed 4 class slots (4*S=128 rows) per SBUF tile.
    OG = -(-CPC // 4)  # output groups

    with tile.TileContext(nc) as tc:
        with (
            tc.tile_pool(name="xp", bufs=1) as xp,
            tc.tile_pool(name="wp", bufs=CPC) as wp,
            tc.tile_pool(name="op", bufs=1) as op,
            tc.tile_pool(name="pp", bufs=4, space="PSUM") as pp,
            tc.tile_pool(name="ap", bufs=1, space="PSUM") as apool,
        ):
            # Single DMA for all of xT: [D, NCOL] -> [128, KT*NCOL].
            xt_t = xp.tile([PT, KT * NCOL], f32, name="x")
            nc.sync.dma_start(
                xt_t[:].rearrange("p (i c) -> p i c", i=KT),
                xt.rearrange("(i p) c -> p i c", p=PT),
            )

            # Per-class weight DMA: [D, U] -> [128, KT*U] (column block i
            # holds rows 128i..128(i+1) of W), double-buffered CPC deep.
            # Class 0 is split into 4 per-i chunks so the first matmul can
            # start as soon as ~1.1 MB (not 1.85 MB) has landed.
            w_ts = []
            for j in range(CPC):
                w_t = wp.tile([PT, KT * U], f32, tag="w", name=f"w{j}")
                if j == 0:
                    for i in range(KT):
                        nc.sync.dma_start(
                            w_t[:, i * U : (i + 1) * U],
                            wt[j, PT * i : PT * (i + 1), :],
                        )
                else:
                    nc.sync.dma_start(
                        w_t[:].rearrange("p (i u) -> p i u", i=KT),
                        wt[j].rearrange("(i p) u -> p i u", p=PT),
                    )
                w_ts.append(w_t)

            ots = [op.tile([PT, U], f32, name=f"o{g}") for g in range(OG)]

            # The LDWEIGHTS half of a matmul only supports one sync wait.
            # Per class, a 1x1 dummy matmul reading only w_t absorbs the
            # weight-DMA wait on the PE engine, so the real matmuls below
            # carry at most the PSUM-reuse wait. The dummies form one
            # accumulation group into the same scratch element so they do
            # not create PSUM WAW waits among themselves.
            scrps = apool.tile([2, 2], f32, name="abs")
            for j in range(CPC):
                # bf16 reinterpret keeps the dummy a single one-pass matmul.
                nc.tensor.matmul(
                    scrps[:],
                    w_ts[j][:, 0:1].bitcast(bf16),
                    w_ts[j][:, 1:2].bitcast(bf16),
                    start=(j == 0),
                    stop=(j == CPC - 1),
                    skip_group_check=True,
                )
                ps = pp.tile([S, U], f32, tag="ps", name=f"ps{j}")
                for i in range(KT):
                    nc.tensor.matmul(
                        ps[:],
                        xt_t[:, i * NCOL + S * j : i * NCOL + S * (j + 1)],
                        w_ts[j][:, i * U : (i + 1) * U],
                        start=(i == 0),
                        stop=(i == KT - 1),
                    )
                # PSUM -> SBUF drain into the packed output tile.
                nc.scalar.copy(ots[j // 4][S * (j % 4) : S * (j % 4) + S, :], ps[:])
                # Store each 128-row group as soon as its last class drains,
                # from the Scalar queue (same engine that produced the data,
                # so the store needs no extra sync and skips the SP queue).
                if j % 4 == 3 or j == CPC - 1:
                    g = j // 4
                    rows = min(PT, NCOL - PT * g)
                    nc.scalar.dma_start(
                        out[PT * g : PT * g + rows, :], ots[g][:rows, :]
                    )
    _split_multi_waits(nc)
    return nc


def kernel(x, classes, kernel, bias):
    global LAST_RESULTS
    x = np.ascontiguousarray(np.asarray(x), dtype=np.float32)
    W = np.ascontiguousarray(np.asarray(kernel), dtype=np.float32)
    bias_np = np.ascontiguousarray(np.asarray(bias), dtype=np.float32)
    cls = np.asarray(classes).reshape(-1).astype(np.int64)

    counts = np.bincount(cls, minlength=C)
    # Fixed column width per class slot; multiple of 8 for DMA alignment.
    S = int(max(32, -(-counts.max() // 8) * 8))
    if S not in _PROG_CACHE:
        _PROG_CACHE[S] = _build_program(S)
    nc = _PROG_CACHE[S]
    NCOL = CPC * S

    order = np.argsort(cls, kind="stable")
    starts = np.zeros(C + 1, np.int64)
    np.cumsum(counts[:C], out=starts[1:])
    rows_by_class = [order[starts[c] : starts[c + 1]] for c in range(C)]

    in_maps = []
    for m in range(NCORES):
        xt_m = np.zeros((D, NCOL), np.float32)
        for j in range(CPC):
            c = m * CPC + j
            if c >= C:
                continue
            r = rows_by_class[c]
            if r.size:
                xt_m[:, S * j : S * j + r.size] = x[r].T
        if (m + 1) * CPC <= C:
            wt_m = W[m * CPC : (m + 1) * CPC]
        else:
            n_real = C - m * CPC
            wt_m = np.concatenate([W[m * CPC : C], W[: CPC - n_real]], axis=0)
        in_maps.append({"xt": xt_m, "wt": wt_m})

    res = run_bass_kernel_spmd(nc, in_maps, list(range(NCORES)))
    LAST_RESULTS = res

    out = np.empty((B, U), np.float32)
    for m in range(NCORES):
        panel = np.asarray(res.results[m]["out"])
        for j in range(CPC):
            c = m * CPC + j
            if c >= C:
                continue
            r = rows_by_class[c]
            if r.size:
                out[r] = panel[S * j : S * j + r.size] + bias_np[c]
    return out
